# revision 1
# baseline (speedup 1.0000x reference)
"""SAGAN-style self-attention block on 8 trn2 NeuronCores.

Full inputs: x [8, 512, 64, 64], w_theta [64, 512], w_phi [64, 512],
w_g [256, 512], w_o [512, 256], gamma scalar.

Sharding: data-parallel over batch — one batch item per core. Each core runs
an identical Bass program over its own x[b]; weights are replicated.

Per-core math (C=512, n=H*W=4096, m=n/4=1024):
  theta = w_theta @ x            [64, 4096]
  phi   = pool2(w_phi @ x)       [64, 1024]
  g     = pool2(w_g @ x)         [256, 1024]
  S^T   = phi^T @ theta          [1024, 4096]   (scores, transposed layout)
  E     = exp(S^T)               (no max-subtraction needed: |S| < ~50)
  Z     = ones^T @ E             [*, 4096]      (row sums, broadcast layout)
  att   = (g @ E) / Z            [256, 4096]
  out   = (gamma*w_o) @ att + x  [512, 4096]

All matmuls run as float32r (full-rate fp32 on the PE at N>=512; tf32-like
input rounding, ~1.5e-4 rel err). The residual add uses unrounded fp32 x.
"""

import time
from contextlib import ExitStack

import numpy as np

import bass_rust
import concourse.bass as bass
import concourse.mybir as mybir
import concourse.tile as tile
from concourse.bass_utils import run_bass_kernel_spmd
from concourse.masks import make_identity

P = 128
C = 512  # channels
C8 = 64  # theta/phi channels
C2 = 256  # g channels
N = 4096  # H*W
M = 1024  # pooled spatial
NS = 8  # n-slices
SL = 512  # n-slice width
MT = 8  # m-tiles of 128
F32 = mybir.dt.float32
F32R = mybir.dt.float32r
AX = mybir.AxisListType
ALU = mybir.AluOpType
ACTF = mybir.ActivationFunctionType


def _pool_view(ap):
    """[p, 512] slice of the conv output -> 5D maxpool view [p, h2, w2, dy, dx].

    Within an n-slice of 512 = 8 image rows: local n = (2*h2+dy)*64 + 2*w2+dx.
    """
    return ap.rearrange("p (h2 dy w2 dx) -> p h2 w2 dy dx", h2=4, dy=2, w2=32, dx=2)


def emit(nc, tc, ctx):
    x_f = nc.dram_tensor("x", [C, N], F32R, kind="ExternalInput")
    wproj = nc.dram_tensor("wproj", [C, 384], F32R, kind="ExternalInput")
    wo = nc.dram_tensor("wo", [C2, C], F32R, kind="ExternalInput")
    out_d = nc.dram_tensor("out", [C, N], F32, kind="ExternalOutput")

    persist = ctx.enter_context(tc.tile_pool(name="persist", bufs=1))

    wpt = persist.tile([P, 4, 384], F32R, name="wpt")
    nc.scalar.dma_start(out=wpt, in_=wproj.ap().rearrange("(k p) o -> p k o", k=4))
    wp = [wpt[:, k, :] for k in range(4)]
    ones_f = persist.tile([P, P], F32)
    nc.vector.memset(ones_f, 1.0)
    ones = persist.tile([P, P], F32R)
    nc.vector.tensor_copy(ones, ones_f)
    ident_f = persist.tile([P, P], F32)
    make_identity(nc, ident_f)
    ident = persist.tile([P, P], F32R)
    nc.vector.tensor_copy(ident, ident_f)

    # score psum pool lives across phases 1+2 so slice-0 scores can start
    # inside phase 1
    spool = ctx.enter_context(tc.tile_pool(name="spsum", bufs=2, space="PSUM"))
    etp = ctx.enter_context(tc.tile_pool(name="et", bufs=3))
    miscp = ctx.enter_context(tc.tile_pool(name="misc", bufs=2))

    # Warm-up work for the otherwise-idle startup window (PE waits ~6us for
    # the first x data): dummy exp preloads the ACT exp table (~2.7us load
    # otherwise lands mid-phase-1 on the cast path), and a burst of matmuls
    # on constant data ramps the PE clock (HAM) before real work arrives.
    actwarm = persist.tile([P, 1], F32)
    nc.scalar.activation(actwarm, ones_f[:, 0:1], ACTF.Exp)
    for wi in range(15):
        wt_ = spool.tile([P, P], F32, name="warm", tag="s0", bufs=1)
        nc.tensor.matmul(wt_, lhsT=ones_f, rhs=ones_f, start=True, stop=True)

    # x loads: slice-major chunks so phase-1 slice 0 unblocks after ~1MB.
    # Tiles are f32r (rounded at DMA time): they feed the projection matmuls
    # directly and the residual adds read them back via bitcast — the ~1e-4
    # relative rounding on the residual is well inside the error budget.
    xf = [persist.tile([P, N], F32R, name=f"xf{cc}") for cc in range(4)]
    for q in range(NS):
        for cc in range(4):
            nc.sync.dma_start(
                out=xf[cc][:, q * SL : (q + 1) * SL],
                in_=x_f[cc * P : (cc + 1) * P, q * SL : (q + 1) * SL],
            )
    wot = []
    for k in range(2):
        t = persist.tile([P, C], F32R, name=f"wot{k}")
        nc.sync.dma_start(out=t, in_=wo[k * P : (k + 1) * P, :])
        wot.append(t)

    theta = persist.tile([C8, N], F32R)
    phi = persist.tile([P, M], F32R)  # [64:128] pooled, [0:64] copy for K rows 0-63
    g = [persist.tile([P, M], F32R, name=f"g{i}") for i in range(2)]
    gT = [persist.tile([P, C2], F32R, name=f"gT{mt}") for mt in range(MT)]

    ET = [[None] * MT for _ in range(NS)]
    FS = [[None] * (MT // 2) for _ in range(NS)]
    TH2 = [None] * NS

    def emit_th2(i):
        nsl = slice(i * SL, (i + 1) * SL)
        t = miscp.tile([P, SL], F32R, name="th2", tag="th2", bufs=2)
        nc.sync.dma_start(out=t[C8:P, :], in_=theta[:, nsl])
        TH2[i] = t

    def emit_score_pair(i, j):
        # m-tiles 2j and 2j+1 run concurrently via row tiling (separate banks)
        nsl = slice(i * SL, (i + 1) * SL)
        for half, mt in enumerate((2 * j, 2 * j + 1)):
            sp = spool.tile([P, SL], F32, name="sp", tag=f"s{half}", bufs=1)
            if half == 0:
                nc.tensor.matmul(
                    sp,
                    lhsT=phi[0:C8, mt * P : (mt + 1) * P],
                    rhs=theta[:, nsl],
                    start=True,
                    stop=True,
                )
            else:
                nc.tensor.matmul(
                    sp,
                    lhsT=phi[C8:P, mt * P : (mt + 1) * P],
                    rhs=TH2[i][C8:P, :],
                    start=True,
                    stop=True,
                    tile_position=(C8, 0),
                )
            et = etp.tile([P, SL], F32R, name="et", tag=f"et{mt}")
            nc.scalar.activation(et, sp, ACTF.Exp)
            ET[i][mt] = et

    def emit_fsums(i):
        # pair-sums on DVE, one slice ahead of the attend stage's Z matmuls
        for j in range(MT // 2):
            fsum = miscp.tile([P, SL], F32R, name="fsum", tag=f"fsum{j}", bufs=2)
            nc.vector.tensor_add(fsum, ET[i][2 * j], ET[i][2 * j + 1])
            FS[i][j] = fsum

    # ---- phase 1: projections + pooling + g transposes -----------------
    with tc.tile_pool(name="ppsum", bufs=2, space="PSUM") as pp, tc.tile_pool(
        name="tpsum", bufs=1, space="PSUM"
    ) as tp:
        for ns in range(NS):
            nsl = slice(ns * SL, (ns + 1) * SL)
            msl = slice(ns * P, (ns + 1) * P)
            xr = [xf[k][:, nsl] for k in range(4)]
            ps = [
                pp.tile(
                    [P, SL], F32, name="pp", tag=f"pp{mt}",
                    bufs=(1 if mt == 0 else 2),
                )
                for mt in range(3)
            ]
            mt_order = (1, 2, 0)
            for mt in mt_order:
                for k in range(4):
                    nc.tensor.matmul(
                        ps[mt],
                        lhsT=wp[k][:, mt * P : (mt + 1) * P],
                        rhs=xr[k],
                        start=(k == 0),
                        stop=(k == 3),
                    )
            # g pools first: with the g-first matmul order their psums are
            # ready first, and they gate this slice's transposes
            for i in range(2):
                nc.vector.tensor_reduce(
                    out=g[i][:, msl],
                    in_=_pool_view(ps[1 + i]),
                    axis=AX.XY,
                    op=ALU.max,
                )
            nc.vector.tensor_reduce(
                out=phi[C8:P, msl],
                in_=_pool_view(ps[0][C8:P, :]),
                axis=AX.XY,
                op=ALU.max,
            )
            nc.sync.dma_start(out=phi[0:C8, msl], in_=phi[C8:P, msl])
            if ns == NS - 1:
                # last slice: keep ACT free so the final score exps (which
                # gate phase-2 entry through the score-slot ring) run sooner
                nc.vector.tensor_copy(out=theta[:, nsl], in_=ps[0][0:C8, :])
            else:
                nc.scalar.copy(out=theta[:, nsl], in_=ps[0][0:C8, :])
            # transpose this slice's pooled g columns into gT[ns]
            for i in range(2):
                t = tp.tile([P, P], F32R, name="tp", tag="tp")
                nc.tensor.transpose(t, g[i][:, msl], ident)
                nc.scalar.copy(out=gT[ns][:, i * P : (i + 1) * P], in_=t)
            if ns == 0:
                emit_th2(0)
            if ns % 2 == 1:
                emit_score_pair(0, ns // 2)

    # ---- phase 2: softmax / attend / project ---------------------------
    with tc.tile_pool(name="qpsum", bufs=2, space="PSUM") as qp:
        def emit_scores(i):
            emit_th2(i)
            for j in range(MT // 2):
                emit_score_pair(i, j)

        def emit_attend(i, lo, w):
            # attend + project + residual for columns [i*SL+lo, i*SL+lo+w)
            nsl = slice(i * SL + lo, i * SL + lo + w)
            esl = slice(lo, lo + w)
            zp = qp.tile([P, w], F32, name="zp", tag="z", bufs=1)
            ap = [qp.tile([P, w], F32, name="ap", tag="a", bufs=3) for _ in range(2)]
            for mt in range(MT):
                st, sp_ = (mt == 0), (mt == MT - 1)
                if mt % 2 == 0:
                    nc.tensor.matmul(
                        zp,
                        lhsT=ones,
                        rhs=FS[i][mt // 2][:, esl],
                        start=st,
                        stop=(mt == MT - 2),
                        skip_group_check=True,
                    )
                for ct in range(2):
                    nc.tensor.matmul(
                        ap[ct],
                        lhsT=gT[mt][:, ct * P : (ct + 1) * P],
                        rhs=ET[i][mt][:, esl],
                        start=st,
                        stop=sp_,
                        skip_group_check=True,
                    )
            rinv = miscp.tile([P, w], F32, name="rinv", tag="rinv")
            nc.vector.reciprocal(rinv, zp)
            att = []
            for ct in range(2):
                t = miscp.tile([P, w], F32R, name="att", tag=f"att{ct}")
                nc.vector.tensor_mul(t, ap[ct], rinv)
                att.append(t)
            for ot in range(4):
                op_ = qp.tile([P, w], F32, name="op", tag="o")
                for ct in range(2):
                    nc.tensor.matmul(
                        op_,
                        lhsT=wot[ct][:, ot * P : (ot + 1) * P],
                        rhs=att[ct],
                        start=(ct == 0),
                        stop=(ct == 1),
                    )
                ob = miscp.tile([P, w], F32, name="ob", tag=f"ob{ot % 2}")
                nc.vector.tensor_add(ob, op_, xf[ot][:, nsl].bitcast(F32))
                nc.sync.dma_start(out=out_d[ot * P : (ot + 1) * P, nsl], in_=ob)

        emit_scores(1)
        emit_fsums(0)
        for i in range(NS):
            if i + 2 < NS:
                emit_scores(i + 2)
            if i + 1 < NS:
                emit_fsums(i + 1)
            emit_attend(i, 0, SL)


def build_nc():
    nc = bass.Bass(target_bir_lowering=False, trn_type="TRN2")
    with tile.TileContext(nc) as tc:
        with ExitStack() as ctx:
            emit(nc, tc, ctx)
    bass_rust.generate_event_semaphores(nc)
    return nc


def kernel(x, w_theta, w_phi, w_g, w_o, gamma):
    x = np.asarray(x, dtype=np.float32)
    B = x.shape[0]
    wproj = np.ascontiguousarray(
        np.concatenate(
            [np.asarray(w_theta).T, np.asarray(w_phi).T, np.asarray(w_g).T], axis=1
        ),
        dtype=np.float32,
    )
    wo_t = np.ascontiguousarray(
        (np.float32(gamma) * np.asarray(w_o)).T, dtype=np.float32
    )

    nc = build_nc()
    in_maps = []
    for b in range(B):
        xb = np.ascontiguousarray(x[b].reshape(C, N))
        in_maps.append({"x": xb, "wproj": wproj, "wo": wo_t})
    # retry: rare transient NRT_EXEC_UNIT_UNRECOVERABLE from stale device
    # state clears on re-execution
    last_err = None
    for attempt in range(3):
        try:
            res = run_bass_kernel_spmd(nc, in_maps, core_ids=list(range(B)))
            break
        except Exception as e:  # noqa: BLE001
            last_err = e
            time.sleep(2.0)
    else:
        raise last_err
    out = np.stack(
        [res.results[b]["out"].reshape(C, 64, 64) for b in range(B)]
    ).astype(np.float32)
    return out



# revision 8
# speedup vs baseline: 1.0087x; 1.0087x over previous
"""SAGAN-style self-attention block on 8 trn2 NeuronCores.

Full inputs: x [8, 512, 64, 64], w_theta [64, 512], w_phi [64, 512],
w_g [256, 512], w_o [512, 256], gamma scalar.

Sharding: data-parallel over batch — one batch item per core. Each core runs
an identical Bass program over its own x[b]; weights are replicated.

Per-core math (C=512, n=H*W=4096, m=n/4=1024):
  theta = w_theta @ x            [64, 4096]
  phi   = pool2(w_phi @ x)       [64, 1024]
  g     = pool2(w_g @ x)         [256, 1024]   (bf16)
  S^T   = phi^T @ theta          [1024, 4096]  (scores, transposed layout)
  E     = exp(S^T)               (bf16; no max-subtraction: |S| < ~50)
  Z     = allreduce_p(sum-tree(E))  (DVE bf16 pair-tree + gpsimd partition
                                     all-reduce -> broadcast row sums)
  att   = (g @ E) / Z            [256, 4096]   (bf16 after normalize)
  out   = (gamma*w_o) @ att + x  [512, 4096]

Matmuls run as float32r (proj/scores; tf32-like rounding) or bf16
(attend/out). The residual add uses unrounded fp32 x. Engine split per
phase-2 slice: PE 32 matmuls, DVE fs-tree+recip+att-muls+1 residual add,
gpsimd (Pool) Z-allreduce+3 residual adds, ACT 8 exps.
"""

import time
from contextlib import ExitStack

import numpy as np

import bass_rust
import concourse.bass as bass
import concourse.bass_isa as bass_isa
import concourse.mybir as mybir
import concourse.tile as tile
from concourse.bass_utils import run_bass_kernel_spmd
from concourse.masks import make_identity

P = 128
C = 512  # channels
C8 = 64  # theta/phi channels
C2 = 256  # g channels
N = 4096  # H*W
M = 1024  # pooled spatial
NS = 8  # n-slices
SL = 512  # n-slice width
MT = 8  # m-tiles of 128
F32 = mybir.dt.float32
F32R = mybir.dt.float32r
BF16 = mybir.dt.bfloat16
AX = mybir.AxisListType
ALU = mybir.AluOpType
ACTF = mybir.ActivationFunctionType


def _pool_view(ap):
    """[p, 512] slice of the conv output -> 5D maxpool view [p, h2, w2, dy, dx].

    Within an n-slice of 512 = 8 image rows: local n = (2*h2+dy)*64 + 2*w2+dx.
    """
    return ap.rearrange("p (h2 dy w2 dx) -> p h2 w2 dy dx", h2=4, dy=2, w2=32, dx=2)


def emit(nc, tc, ctx):
    x_f = nc.dram_tensor("x", [C, N], F32R, kind="ExternalInput")
    wproj = nc.dram_tensor("wproj", [C, 384], F32R, kind="ExternalInput")
    wo = nc.dram_tensor("wo", [C2, C], BF16, kind="ExternalInput")
    out_d = nc.dram_tensor("out", [C, N], F32, kind="ExternalOutput")

    persist = ctx.enter_context(tc.tile_pool(name="persist", bufs=1))

    # weights first on the scalar (ACT) DMA queue: chunk k=0 lands early so
    # the first projection matmul only waits on it + the first x chunk
    wpt = persist.tile([P, 4, 384], F32R, name="wpt")
    for k in range(4):
        nc.scalar.dma_start(out=wpt[:, k, :], in_=wproj[k * P : (k + 1) * P, :])
    wp = [wpt[:, k, :] for k in range(4)]
    wot = persist.tile([P, 2, C], BF16, name="wot")
    for ct in range(2):
        nc.scalar.dma_start(
            out=wot[:, ct, :], in_=wo[ct * P : (ct + 1) * P, :]
        )

    ones_f = persist.tile([P, P], F32)
    nc.vector.memset(ones_f, 1.0)
    ident_f = persist.tile([P, P], F32)
    make_identity(nc, ident_f)
    ident = persist.tile([P, P], F32R)
    nc.vector.tensor_copy(ident, ident_f)

    # score psum pool lives across both phases (slice 0/1 scores start in
    # phase 1)
    spool = ctx.enter_context(tc.tile_pool(name="spsum", bufs=3, space="PSUM"))
    etp = ctx.enter_context(tc.tile_pool(name="et", bufs=3))
    fsp = ctx.enter_context(tc.tile_pool(name="fs", bufs=2))
    miscp = ctx.enter_context(tc.tile_pool(name="misc", bufs=2))

    # Warm-up for the otherwise-idle startup window (PE waits ~4us for the
    # first weight+x data): dummy exp preloads the ACT exp table, and a burst
    # of matmuls on constant data ramps the PE clock (HAM).
    actwarm = persist.tile([P, 1], F32)
    nc.scalar.activation(actwarm, ones_f[:, 0:1], ACTF.Exp)
    for wi in range(6):
        wt_ = spool.tile([P, P], F32, name="warm", tag="s0", bufs=1)
        nc.tensor.matmul(wt_, lhsT=ones_f, rhs=ones_f, start=True, stop=True)

    # x loads: slice-major chunks on the sync (SP) queue so phase-1 slice 0
    # unblocks after ~1MB. Tiles are f32r (rounded at DMA time): they feed
    # the projection matmuls directly and the residual adds read them back
    # via bitcast.
    xf = [persist.tile([P, N], F32R, name=f"xf{cc}") for cc in range(4)]
    for q in range(NS):
        for cc in range(4):
            nc.sync.dma_start(
                out=xf[cc][:, q * SL : (q + 1) * SL],
                in_=x_f[cc * P : (cc + 1) * P, q * SL : (q + 1) * SL],
            )

    theta = persist.tile([C8, N], F32R)
    phi = persist.tile([C8, M], F32R)
    g = [persist.tile([P, M], F32R, name=f"g{i}") for i in range(2)]
    gT = [persist.tile([P, C2], BF16, name=f"gT{mt}") for mt in range(MT)]

    ET = [[None] * MT for _ in range(NS)]
    FS1 = [[None] * 4 for _ in range(NS)]  # pair sums
    ZB = [None] * NS  # broadcast row sums
    ATT = [[None, None] for _ in range(NS)]
    score_done = [[False] * MT for _ in range(NS)]
    fs1_done = [[False] * 4 for _ in range(NS)]

    def emit_score(i, mt):
        # S^T tile [m 128, n 512] = phi_mt^T @ theta_i  (K = 64 channels)
        sp = spool.tile([P, SL], F32, name="sp", tag=f"s{mt % 3}", bufs=1)
        nc.tensor.matmul(
            sp,
            lhsT=phi[:, mt * P : (mt + 1) * P],
            rhs=theta[:, i * SL : (i + 1) * SL],
            start=True,
            stop=True,
            skip_group_check=True,
        )
        et = etp.tile([P, SL], BF16, name="et", tag=f"et{mt}")
        nc.scalar.activation(et, sp, ACTF.Exp)
        ET[i][mt] = et
        score_done[i][mt] = True

    def emit_fs1(i, j):
        t = fsp.tile([P, SL], BF16, name="fs1", tag=f"f{j}")
        nc.vector.tensor_add(t, ET[i][2 * j], ET[i][2 * j + 1])
        FS1[i][j] = t
        fs1_done[i][j] = True

    def emit_fs_rest(i):
        # finish the pair-sum tree and hand the total to the Pool engine for
        # the cross-partition broadcast sum
        for j in range(4):
            if not fs1_done[i][j]:
                emit_fs1(i, j)
        h0 = fsp.tile([P, SL], BF16, name="fs2", tag="h0")
        h1 = fsp.tile([P, SL], BF16, name="fs2", tag="h1")
        nc.vector.tensor_add(h0, FS1[i][0], FS1[i][1])
        nc.vector.tensor_add(h1, FS1[i][2], FS1[i][3])
        hh = fsp.tile([P, SL], BF16, name="fs3", tag="hh")
        nc.vector.tensor_add(hh, h0, h1)
        zb = fsp.tile([P, SL], F32, name="zb", tag="zb")
        nc.gpsimd.partition_all_reduce(zb, hh, P, bass_isa.ReduceOp.add)
        ZB[i] = zb

    def emit_scores_full(i):
        for mt in range(MT):
            if not score_done[i][mt]:
                emit_score(i, mt)

    def emit_ap(qp, i, lo, w):
        esl = slice(lo, lo + w)
        ap = [
            qp.tile([P, w], F32, name="ap", tag=f"a{ct}", bufs=1)
            for ct in range(2)
        ]
        for mt in range(MT):
            st, sp_ = (mt == 0), (mt == MT - 1)
            for ct in range(2):
                nc.tensor.matmul(
                    ap[ct],
                    lhsT=gT[mt][:, ct * P : (ct + 1) * P],
                    rhs=ET[i][mt][:, esl],
                    start=st,
                    stop=sp_,
                    skip_group_check=True,
                )
        return ap

    def emit_norm(i, ap, lo, w):
        esl = slice(lo, lo + w)
        rinv = miscp.tile([P, w], F32, name="rinv", tag="rinv")
        nc.vector.reciprocal(rinv, ZB[i][:, esl])
        att = []
        for ct in range(2):
            t = miscp.tile([P, w], BF16, name="att", tag=f"att{ct}")
            nc.vector.tensor_mul(t, ap[ct], rinv)
            att.append(t)
        ATT[i] = att

    def emit_out(qp, i, lo, w, queues=None):
        # project + residual + store for columns [i*SL+lo, i*SL+lo+w)
        nsl = slice(i * SL + lo, i * SL + lo + w)
        att = ATT[i]
        for ot in range(4):
            op_ = qp.tile([P, w], F32, name="op", tag="o", bufs=3)
            for ct in range(2):
                nc.tensor.matmul(
                    op_,
                    lhsT=wot[:, ct, ot * P : (ot + 1) * P],
                    rhs=att[ct],
                    start=(ct == 0),
                    stop=(ct == 1),
                    skip_group_check=True,
                )
            ob = miscp.tile([P, w], F32, name="ob", tag=f"ob{ot}")
            eng = nc.vector if ot == 0 else nc.gpsimd
            eng.tensor_add(ob, op_, xf[ot][:, nsl].bitcast(F32))
            q = queues[ot % len(queues)] if queues else nc.sync
            q.dma_start(out=out_d[ot * P : (ot + 1) * P, nsl], in_=ob)

    # phase-1 score pull-in: fill PE gaps left by the x-DMA cadence with
    # slice-0/1 score matmuls (their exps + pair sums trail on ACT/DVE)
    pull = [(0, 0), (0, 1)]
    for mt in range(1, 7):
        pull += [(1, mt - 1), (0, mt + 1)]
    pull += [(1, 6)]
    pulled = 0

    def pump_scores(ns, budget):
        nonlocal pulled
        done = 0
        while pulled < len(pull) and done < budget:
            i, mt = pull[pulled]
            if i > ns - 1 or mt > ns - 1:
                break
            emit_score(i, mt)
            if mt % 2 == 1:
                emit_fs1(i, mt // 2)
            pulled += 1
            done += 1

    # ---- phase 1: projections + pooling + g transposes -----------------
    with tc.tile_pool(name="ppsum", bufs=1, space="PSUM") as pp, tc.tile_pool(
        name="tpsum", bufs=1, space="PSUM"
    ) as tp:
        for ns in range(NS):
            nsl = slice(ns * SL, (ns + 1) * SL)
            msl = slice(ns * P, (ns + 1) * P)
            xr = [xf[k][:, nsl] for k in range(4)]
            ps = [
                pp.tile([P, SL], F32, name="pp", tag=f"pp{mt}", bufs=1)
                for mt in range(3)
            ]
            # g-first matmul order: their psums are ready first and gate this
            # slice's transposes
            for mt in (1, 2, 0):
                for k in range(4):
                    nc.tensor.matmul(
                        ps[mt],
                        lhsT=wp[k][:, mt * P : (mt + 1) * P],
                        rhs=xr[k],
                        start=(k == 0),
                        stop=(k == 3),
                    )
            for i in range(2):
                nc.vector.tensor_reduce(
                    out=g[i][:, msl],
                    in_=_pool_view(ps[1 + i]),
                    axis=AX.XY,
                    op=ALU.max,
                )
            nc.vector.tensor_reduce(
                out=phi[:, msl],
                in_=_pool_view(ps[0][C8:P, :]),
                axis=AX.XY,
                op=ALU.max,
            )
            nc.scalar.copy(out=theta[:, nsl], in_=ps[0][0:C8, :])
            # transpose this slice's pooled g columns into gT[ns]
            for i in range(2):
                t = tp.tile([P, P], F32R, name="tp", tag="tp")
                nc.tensor.transpose(t, g[i][:, msl], ident)
                nc.scalar.copy(out=gT[ns][:, i * P : (i + 1) * P], in_=t)
            pump_scores(ns, 2)

    # ---- phase 2: softmax / attend / project ---------------------------
    with tc.tile_pool(name="qpsum", bufs=1, space="PSUM") as qp:
        pump_scores(NS, 99)  # leftovers with mt < 7
        emit_scores_full(0)
        emit_scores_full(1)
        emit_fs_rest(0)
        for i in range(NS):
            last = i == NS - 1
            if i >= 1:
                emit_out(qp, i - 1, 0, SL)
            if not last:
                ap = emit_ap(qp, i, 0, SL)
                if i + 2 < NS:
                    emit_scores_full(i + 2)
                emit_fs_rest(i + 1)
                emit_norm(i, ap, 0, SL)
            else:
                # drain fast: narrow trailing chunks so the final
                # normalize/project/store pipeline is short
                for lo, w in ((0, 256), (256, 128), (384, 128)):
                    ap = emit_ap(qp, i, lo, w)
                    emit_norm(i, ap, lo, w)
                    emit_out(qp, i, lo, w, queues=[nc.sync, nc.scalar])


def build_nc():
    nc = bass.Bass(target_bir_lowering=False, trn_type="TRN2")
    with tile.TileContext(nc) as tc:
        with ExitStack() as ctx:
            emit(nc, tc, ctx)
    bass_rust.generate_event_semaphores(nc)
    return nc


def kernel(x, w_theta, w_phi, w_g, w_o, gamma):
    import ml_dtypes

    x = np.asarray(x, dtype=np.float32)
    B = x.shape[0]
    wproj = np.ascontiguousarray(
        np.concatenate(
            [np.asarray(w_theta).T, np.asarray(w_phi).T, np.asarray(w_g).T], axis=1
        ),
        dtype=np.float32,
    )
    wo_t = np.ascontiguousarray(
        (np.float32(gamma) * np.asarray(w_o)).T.astype(ml_dtypes.bfloat16)
    )

    nc = build_nc()
    in_maps = []
    for b in range(B):
        xb = np.ascontiguousarray(x[b].reshape(C, N))
        in_maps.append({"x": xb, "wproj": wproj, "wo": wo_t})
    # retry: rare transient NRT_EXEC_UNIT_UNRECOVERABLE from stale device
    # state clears on re-execution
    last_err = None
    for attempt in range(3):
        try:
            res = run_bass_kernel_spmd(nc, in_maps, core_ids=list(range(B)))
            break
        except Exception as e:  # noqa: BLE001
            last_err = e
            time.sleep(2.0)
    else:
        raise last_err
    out = np.stack(
        [res.results[b]["out"].reshape(C, 64, 64) for b in range(B)]
    ).astype(np.float32)
    return out
